# revision 25
# baseline (speedup 1.0000x reference)
"""Multi-head self-attention on Trainium2, 8-core SPMD — pair-pipelined v5.

Problem: x[2,2048,1024] -> torch-style MHSA (16 heads, head_dim 64) -> [2,2048,1024]

Sharding (data + tensor parallel): 8 cores = 2 batches x 4 head-groups.
Each core owns one batch and 4 heads (256 channels). It computes Q/K/V
projections for its channels, attention for its 4 heads, and the
out-projection with its 256 rows of Wo, producing a partial [S, E] output.
The host sums the 4 head-group partials per batch and adds bo.

Pipelining: the attention inner loop is ACT(exp)-bound (~1 elem/lane/cycle),
so the core's 4 heads are processed as two PAIRS and the PE work of pair 1's
projections fills the exp-bound window of pair 0's attention:

  [K(p0) racing xT DMA, Q(p0,qb0)]
  W1: attn(p0) | V(p0) per-kt, Q-rest(p0), K(p1), V(p1) per-kt, Q(p1)
  W2: attn(p1) | out-proj(all 4 heads) lagged one query block
  tail: out-proj(last qb)

Design notes:
  * Attention runs entirely transposed so no on-chip transposes are needed:
        scoresT[k,q] = lhsT(K^T[dh,k]) x rhs(Q^T[dh,q]),  contraction dh=64,
        two heads packed in the PE array rows via auto-derived row groups
        (lhsT base partitions 0/64; explicit tile_position measured SLOWER),
        attnT = exp(scoresT)  (max-subtraction skipped: |scores| < 3),
        outT[dh,q] (+denominator row) = lhsT([V_h | ones][k,65]) x rhs(attnT).
  * Softmax normalization deferred: outT rows scaled by the reciprocal
    denominator (partition-broadcast via a DRAM bounce) before out-proj.
  * 1/sqrt(head_dim) folded into Wq/bq on the host.
  * Output partial written as bf16 to halve store traffic; host accumulates
    in float64.
  * PSUM: scores 2x2 banks + pv 2 + (proj 2 in W1 | outproj 2 in W2) = 8.
"""

from contextlib import ExitStack

import numpy as np

import concourse.bacc as bacc
import concourse.mybir as mybir
import concourse.tile as tile

P = 128
DH = 64  # head dim
F32 = mybir.dt.float32

# full-size problem constants
FULL_B = 2
FULL_S = 2048
FULL_E = 1024
FULL_H = 16
HPC = 4          # heads per core (one batch per core)
NPAIR = HPC // 2
N_CORES = 8
HD = HPC * DH    # projected channels per core (256)
PW = 2 * DH      # channels per head pair (128)
VN = DH + 1      # V columns per head incl. ones column (65)


def build_nc(S=FULL_S, E=FULL_E, reps=1, mm_dtype=None, att_dtype=None,
             dma_engine="sync"):
    """Build the single-core Bass program (same program on all 8 cores)."""
    assert S % P == 0 and E % P == 0
    EK = E // P              # contraction tiles for projections (8)
    SK = S // P              # key tiles for attention (16)
    QB = min(512, S)         # attention query block (per head)
    NQB = S // QB            # 4
    MMW = min(512, S)        # proj matmul moving width
    NMB = S // MMW           # proj psum passes per pair (4)
    EMW = min(512, E)        # out-projection matmul moving width
    NEB = E // EMW           # 2

    MD = mm_dtype if mm_dtype is not None else mybir.dt.bfloat16
    AD = att_dtype if att_dtype is not None else MD

    nc = bacc.Bacc(trn_type="TRN2", target_bir_lowering=False, debug=False)
    dmae = lambda: getattr(nc, dma_engine)

    xT = nc.declare_dram_parameter("xT", [E, S], MD, isOutput=False)
    wq = nc.declare_dram_parameter("wq", [E, HD], MD, isOutput=False)
    wk = nc.declare_dram_parameter("wk", [E, HD], MD, isOutput=False)
    wv = nc.declare_dram_parameter("wv", [E, HD], MD, isOutput=False)
    wo = nc.declare_dram_parameter("wo", [HD, E], AD, isOutput=False)
    bq = nc.declare_dram_parameter("bq", [P, NPAIR], F32, isOutput=False)
    bk = nc.declare_dram_parameter("bk", [P, NPAIR], F32, isOutput=False)
    bvb = nc.declare_dram_parameter("bvb", [P, HD], F32, isOutput=False)
    out = nc.declare_dram_parameter("out", [S, E], AD, isOutput=True)
    rcp_dram = nc.dram_tensor("rcp_scratch", [HPC, S], F32)

    Exp = mybir.ActivationFunctionType.Exp
    Add = mybir.AluOpType.add

    xT_t = xT.rearrange("(kt p) s -> kt p s", p=P)

    with ExitStack() as ctx:
        tc = ctx.enter_context(tile.TileContext(nc))
        for _rep in range(reps):
            rctx = ctx.enter_context(ExitStack())
            const = rctx.enter_context(tc.tile_pool(name="const", bufs=1))
            proj = rctx.enter_context(tc.tile_pool(name="proj", bufs=1))
            attn_out = rctx.enter_context(tc.tile_pool(name="attn_out", bufs=1))

            bq_sb = const.tile([P, NPAIR], F32)
            dmae().dma_start(out=bq_sb[:], in_=bq[:, :])
            bk_sb = const.tile([P, NPAIR], F32)
            dmae().dma_start(out=bk_sb[:], in_=bk[:, :])
            bv_sb = const.tile([P, HD], F32)
            dmae().dma_start(out=bv_sb[:], in_=bvb[:, :])
            wo_sb = const.tile([DH, HPC, E], AD)
            dmae().dma_start(
                out=wo_sb[:], in_=wo.rearrange("(h p) e -> p h e", p=DH))

            # persistent activation tensors, per head pair
            qt_sb = [proj.tile([P, S], MD, name=f"qt{p}", tag=f"qt{p}")
                     for p in range(NPAIR)]
            kt_sb = [proj.tile([P, S], MD, name=f"kt{p}", tag=f"kt{p}")
                     for p in range(NPAIR)]
            vhat = [proj.tile([P, SK, 2 * VN], AD, name=f"vh{p}", tag=f"vh{p}")
                    for p in range(NPAIR)]
            outT = attn_out.tile([DH, HPC, S], AD)

            # ---- attention-phase pools (live through W1 + W2; opened
            # first so the projection pools can close before op_ps opens) ----
            actx = ExitStack()
            sc_ps = actx.enter_context(
                tc.tile_pool(name="sc_ps", bufs=2, space="PSUM"))
            pv_ps = actx.enter_context(
                tc.tile_pool(name="pv_ps", bufs=2, space="PSUM"))
            at_pool = actx.enter_context(tc.tile_pool(name="at", bufs=10))
            rcp_pool = actx.enter_context(tc.tile_pool(name="rcp", bufs=3))
            rb_pool = actx.enter_context(tc.tile_pool(name="rb", bufs=3))

            # ---- projection-phase pools (live through W1) ----
            pctx = ExitStack()
            xt_pool = pctx.enter_context(tc.tile_pool(name="xt", bufs=1))
            w_pool = pctx.enter_context(tc.tile_pool(name="wghts", bufs=1))
            prj_ps = pctx.enter_context(
                tc.tile_pool(name="prj_ps", bufs=2, space="PSUM"))

            wq_sb = w_pool.tile([P, EK, HD], MD)
            dmae().dma_start(
                out=wq_sb[:], in_=wq.rearrange("(kt p) n -> p kt n", p=P))
            wk_sb = w_pool.tile([P, EK, HD], MD)
            dmae().dma_start(
                out=wk_sb[:], in_=wk.rearrange("(kt p) n -> p kt n", p=P))
            wv_sb = w_pool.tile([P, EK, HD], MD)
            dmae().dma_start(
                out=wv_sb[:], in_=wv.rearrange("(kt p) n -> p kt n", p=P))

            xt = xt_pool.tile([P, EK, S], MD)

            def load_xt():
                # per-kt descriptors so the K projection can race the DMA
                for kt in range(EK):
                    dmae().dma_start(out=xt[:, kt, :], in_=xT_t[kt])

            def emit_qk_proj(p, w_sb, b_sb, dst, mbs):
                # psum[ch=128, MMW] = W_pair^T x xT block; bias on copy-out
                csl = slice(p * PW, (p + 1) * PW)
                for mb in mbs:
                    ssl = slice(mb * MMW, (mb + 1) * MMW)
                    ps = prj_ps.tile([P, MMW], F32, tag="prj")
                    for kt in range(EK):
                        nc.tensor.matmul(
                            ps[:],
                            lhsT=w_sb[:, kt, csl],
                            rhs=xt[:, kt, ssl],
                            start=(kt == 0),
                            stop=(kt == EK - 1),
                        )
                    nc.vector.tensor_scalar(
                        out=dst[:, ssl], in0=ps[:],
                        scalar1=b_sb[:, p:p + 1], scalar2=None, op0=Add,
                    )

            def emit_k_proj_ktouter(p, mb0s):
                # kt-outer K-projection: consumes xt tiles in DMA order
                csl = slice(p * PW, (p + 1) * PW)
                for mb0 in mb0s:
                    kps = [prj_ps.tile([P, MMW], F32, name=f"kp{mb0 + i}",
                                       tag="prj") for i in range(2)]
                    for kt in range(EK):
                        for i in range(2):
                            mb = mb0 + i
                            nc.tensor.matmul(
                                kps[i][:],
                                lhsT=wk_sb[:, kt, csl],
                                rhs=xt[:, kt, mb * MMW:(mb + 1) * MMW],
                                start=(kt == 0),
                                stop=(kt == EK - 1),
                            )
                    for i in range(2):
                        mb = mb0 + i
                        nc.vector.tensor_scalar(
                            out=kt_sb[p][:, mb * MMW:(mb + 1) * MMW],
                            in0=kps[i][:],
                            scalar1=bk_sb[:, p:p + 1], scalar2=None, op0=Add,
                        )

            def emit_v_proj(p, sts):
                # psum[s_tile=128, 128] = xT_tile^T x Wv_pair -> vhat columns
                csl = slice(p * PW, (p + 1) * PW)
                for st in sts:
                    ps = prj_ps.tile([P, PW], F32, tag="prj")
                    for kt in range(EK):
                        nc.tensor.matmul(
                            ps[:],
                            lhsT=xt[:, kt, st * P:(st + 1) * P],
                            rhs=wv_sb[:, kt, csl],
                            start=(kt == 0),
                            stop=(kt == EK - 1),
                        )
                    vdst = vhat[p][:, st, :].rearrange(
                        "p (h c) -> p h c", c=VN)[:, :, 0:DH]
                    nc.vector.tensor_add(
                        out=vdst,
                        in0=ps[:].rearrange("p (h c) -> p h c", c=DH),
                        in1=bv_sb[:, csl].rearrange("p (h c) -> p h c", c=DH),
                    )

            def emit_attn_qb(p, qb, extra_kt=None):
                qsl = slice(qb * QB, (qb + 1) * QB)
                pv0 = pv_ps.tile([VN, QB], F32, tag="pv")
                pv1 = pv_ps.tile([VN, QB], F32, tag="pv")

                def emit_pv(kt, at):
                    for hl, pv, asl in (
                        (0, pv0, slice(0, QB)),
                        (1, pv1, slice(QB, 2 * QB)),
                    ):
                        nc.tensor.matmul(
                            pv[:, :],
                            lhsT=vhat[p][:, kt, hl * VN:(hl + 1) * VN],
                            rhs=at[:, asl],
                            start=(kt == 0),
                            stop=(kt == SK - 1),
                        )

                # PV lags one kt so the in-order PE stream never parks on the
                # freshly-issued exp: sc(kt+1) issues right after pv(kt-1),
                # keeping the ACT queue fed regardless of semaphore latency.
                pending = None
                for kt in range(SK):
                    if extra_kt is not None:
                        extra_kt(kt)
                    ksl = slice(kt * P, (kt + 1) * P)
                    sc = sc_ps.tile([P, 2 * QB], F32, tag="sc")
                    nc.tensor.matmul(
                        sc[:, 0:QB],
                        lhsT=kt_sb[p][0:DH, ksl],
                        rhs=qt_sb[p][0:DH, qsl],
                        start=True, stop=True,
                    )
                    nc.tensor.matmul(
                        sc[:, QB:2 * QB],
                        lhsT=kt_sb[p][DH:P, ksl],
                        rhs=qt_sb[p][DH:P, qsl],
                        start=True, stop=True,
                    )
                    at = at_pool.tile([P, 2 * QB], AD, tag="at")
                    nc.scalar.activation(out=at[:], in_=sc[:], func=Exp)
                    if pending is not None:
                        emit_pv(*pending)
                    pending = (kt, at)
                emit_pv(*pending)
                for hl, pv in ((0, pv0), (1, pv1)):
                    h = 2 * p + hl
                    rcp = rcp_pool.tile([VN, QB], F32, tag="rcp")
                    nc.vector.reciprocal(out=rcp[DH:VN, :], in_=pv[DH:VN, :])
                    nc.vector.tensor_copy(
                        out=outT[0:DH, h, qsl], in_=pv[0:DH, :])
                    dmae().dma_start(
                        out=rcp_dram[h:h + 1, qsl],
                        in_=rcp[DH:VN, :],
                    )
                    rb = rb_pool.tile([DH, QB], F32, tag="rb")
                    dmae().dma_start(
                        out=rb[:],
                        in_=rcp_dram[h:h + 1, qsl].to_broadcast((DH, QB)),
                    )
                    nc.vector.tensor_mul(
                        out=outT[:, h, qsl], in0=outT[:, h, qsl],
                        in1=rb[:],
                    )

            # ====== prologue: K(p0) kt-outer racing the xt DMA, Q(p0,qb0) ==
            load_xt()
            for p in range(NPAIR):
                ones_ap = vhat[p][:].rearrange(
                    "p st (h c) -> p st h c", c=VN)[:, :, :, DH:VN]
                nc.vector.memset(ones_ap, 1.0)
            # Q(qb0) between the K groups: scores(qb0, kt<8) need only K
            # mb0/mb1, so the first exp can fire while K-group-2 still runs
            emit_k_proj_ktouter(0, [0])
            emit_qk_proj(0, wq_sb, bq_sb, qt_sb[0], [0])
            emit_k_proj_ktouter(0, [2])

            def v1_first_half(kt):
                if kt < SK // 2:
                    emit_v_proj(1, [kt])

            def v1_second_half(kt):
                if kt >= SK // 2:
                    emit_v_proj(1, [kt])

            # ========== W1: attn(p0) + projections for p0-rest and p1 ======
            for qb in range(NQB):
                if qb == 0:
                    emit_attn_qb(0, qb, extra_kt=lambda kt: emit_v_proj(0, [kt]))
                    emit_qk_proj(0, wq_sb, bq_sb, qt_sb[0], range(1, NMB))
                elif qb == 1:
                    emit_attn_qb(0, qb)
                    emit_qk_proj(1, wk_sb, bk_sb, kt_sb[1], range(NMB))
                elif qb == 2:
                    emit_attn_qb(0, qb, extra_kt=v1_first_half)
                else:
                    emit_attn_qb(0, qb)
                    emit_qk_proj(1, wq_sb, bq_sb, qt_sb[1], range(NMB))

            # ========== W2: attn(p1) + out-proj (all heads, lagged) ========
            # V(p1) second half projects inside W2-qb0 (no out-proj there
            # yet); the projection pools close right after so op_ps can open.
            emit_attn_qb(1, 0, extra_kt=v1_second_half)
            pctx.close()

            octx = ExitStack()
            op_ps = octx.enter_context(
                tc.tile_pool(name="op_ps", bufs=2, space="PSUM"))
            ob_pool = octx.enter_context(tc.tile_pool(name="ob", bufs=3))

            def emit_op_qb(qb):
                # one ob staging tile + one store DMA per qb; contraction
                # over all 4 heads (K=64 each) accumulated in psum
                ob = ob_pool.tile([P, QB // P, E], AD, tag="ob")
                for m in range(QB // P):
                    msl = slice(qb * QB + m * P, qb * QB + (m + 1) * P)
                    for nb in range(NEB):
                        esl = slice(nb * EMW, (nb + 1) * EMW)
                        ps = op_ps.tile([P, EMW], F32, tag="op")
                        for h in range(HPC):
                            nc.tensor.matmul(
                                ps[:],
                                lhsT=outT[:, h, msl],
                                rhs=wo_sb[:, h, esl],
                                start=(h == 0),
                                stop=(h == HPC - 1),
                            )
                        nc.vector.tensor_copy(out=ob[:, m, esl], in_=ps[:])
                dst = out[qb * QB:(qb + 1) * QB, :].rearrange(
                    "(m p) e -> p m e", p=P)
                dmae().dma_start(out=dst, in_=ob[:])

            for qb in range(1, NQB):
                emit_attn_qb(1, qb)
                emit_op_qb(qb - 1)
            emit_op_qb(NQB - 1)

            octx.close()
            actx.close()
            rctx.close()

    nc.compile()
    return nc


def make_in_maps(x, Wq, bq, Wk, bk, Wv, bv, Wo, n_cores=N_CORES,
                 mm_np_dtype=np.float32, wo_np_dtype=None):
    """Host-side sharding: per-core input dict list."""
    x = np.asarray(x, dtype=np.float32)
    B = x.shape[0]
    groups = n_cores // B
    scale = 1.0 / np.sqrt(np.float32(DH))
    if wo_np_dtype is None:
        wo_np_dtype = mm_np_dtype
    in_maps = []
    for c in range(n_cores):
        b, g = divmod(c, groups)
        hs = slice(g * HD, (g + 1) * HD)
        bq_s = (np.asarray(bq)[hs] * scale).astype(np.float32)
        bk_s = np.asarray(bk)[hs].astype(np.float32)
        in_maps.append({
            "xT": np.ascontiguousarray(x[b].T).astype(mm_np_dtype),
            "wq": np.ascontiguousarray(
                np.asarray(Wq)[:, hs] * scale).astype(mm_np_dtype),
            "wk": np.ascontiguousarray(np.asarray(Wk)[:, hs]).astype(mm_np_dtype),
            "wv": np.ascontiguousarray(np.asarray(Wv)[:, hs]).astype(mm_np_dtype),
            "wo": np.ascontiguousarray(np.asarray(Wo)[hs, :]).astype(wo_np_dtype),
            "bq": np.ascontiguousarray(bq_s.reshape(-1, P).T),
            "bk": np.ascontiguousarray(bk_s.reshape(-1, P).T),
            "bvb": np.ascontiguousarray(
                np.broadcast_to(np.asarray(bv)[hs], (P, HD))
            ),
        })
    return in_maps


_NC_CACHE = {}


def _get_nc():
    if "nc" not in _NC_CACHE:
        _NC_CACHE["nc"] = build_nc(mm_dtype=mybir.dt.bfloat16,
                                   att_dtype=mybir.dt.bfloat16)
    return _NC_CACHE["nc"]


def kernel(x, Wq, bq, Wk, bk, Wv, bv, Wo, bo, _trace=False, _trace_kwargs=None):
    import ml_dtypes
    from concourse.bass_utils import run_bass_kernel_spmd

    x = np.asarray(x, dtype=np.float32)
    B, S, E = x.shape
    nc = _get_nc()
    in_maps = make_in_maps(x, Wq, bq, Wk, bk, Wv, bv, Wo,
                           mm_np_dtype=ml_dtypes.bfloat16,
                           wo_np_dtype=ml_dtypes.bfloat16)
    res = run_bass_kernel_spmd(
        nc, in_maps, list(range(N_CORES)),
        trace=_trace, **(_trace_kwargs or {}),
    )
    groups = N_CORES // B
    full = np.zeros((B, S, E), dtype=np.float64)
    for c in range(N_CORES):
        full[c // groups] += np.asarray(res.results[c]["out"], dtype=np.float64)
    full += np.asarray(bo, dtype=np.float64)
    out = full.astype(np.float32)
    if _trace:
        return out, res
    return out


# revision 27
# speedup vs baseline: 2.0139x; 2.0139x over previous
"""Multi-head self-attention on Trainium2, 8-core SPMD — pair-pipelined v5.

Problem: x[2,2048,1024] -> torch-style MHSA (16 heads, head_dim 64) -> [2,2048,1024]

Sharding (data + tensor parallel): 8 cores = 2 batches x 4 head-groups.
Each core owns one batch and 4 heads (256 channels). It computes Q/K/V
projections for its channels, attention for its 4 heads, and the
out-projection with its 256 rows of Wo, producing a partial [S, E] output.
The host sums the 4 head-group partials per batch and adds bo.

Pipelining: the attention inner loop is ACT(exp)-bound (~1 elem/lane/cycle),
so the core's 4 heads are processed as two PAIRS and the PE work of pair 1's
projections fills the exp-bound window of pair 0's attention:

  [K(p0) racing xT DMA, Q(p0,qb0)]
  W1: attn(p0) | V(p0) per-kt, Q-rest(p0), K(p1), V(p1) per-kt, Q(p1)
  W2: attn(p1) | out-proj(all 4 heads) lagged one query block
  tail: out-proj(last qb)

Design notes:
  * Attention runs entirely transposed so no on-chip transposes are needed:
        scoresT[k,q] = lhsT(K^T[dh,k]) x rhs(Q^T[dh,q]),  contraction dh=64,
        two heads packed in the PE array rows via auto-derived row groups
        (lhsT base partitions 0/64; explicit tile_position measured SLOWER),
        attnT = exp(scoresT)  (max-subtraction skipped: |scores| < 3),
        outT[dh,q] (+denominator row) = lhsT([V_h | ones][k,65]) x rhs(attnT).
  * Softmax normalization deferred: outT rows scaled by the reciprocal
    denominator (partition-broadcast via a DRAM bounce) before out-proj.
  * 1/sqrt(head_dim) folded into Wq/bq on the host.
  * Output partial written as bf16 to halve store traffic; host accumulates
    in float64.
  * PSUM: scores 2x2 banks + pv 2 + (proj 2 in W1 | outproj 2 in W2) = 8.
"""

from contextlib import ExitStack

import numpy as np

import concourse.bacc as bacc
import concourse.mybir as mybir
import concourse.tile as tile

P = 128
DH = 64  # head dim
F32 = mybir.dt.float32

# full-size problem constants
FULL_B = 2
FULL_S = 2048
FULL_E = 1024
FULL_H = 16
HPC = 4          # heads per core (one batch per core)
NPAIR = HPC // 2
N_CORES = 8
HD = HPC * DH    # projected channels per core (256)
PW = 2 * DH      # channels per head pair (128)
VN = DH + 1      # V columns per head incl. ones column (65)


def build_nc(S=FULL_S, E=FULL_E, reps=1, mm_dtype=None, att_dtype=None,
             dma_engine="sync"):
    """Build the single-core Bass program (same program on all 8 cores)."""
    assert S % P == 0 and E % P == 0
    EK = E // P              # contraction tiles for projections (8)
    SK = S // P              # key tiles for attention (16)
    QB = min(512, S)         # attention query block (per head)
    NQB = S // QB            # 4
    MMW = min(512, S)        # proj matmul moving width
    NMB = S // MMW           # proj psum passes per pair (4)
    EMW = min(512, E)        # out-projection matmul moving width
    NEB = E // EMW           # 2

    MD = mm_dtype if mm_dtype is not None else mybir.dt.bfloat16
    AD = att_dtype if att_dtype is not None else MD

    nc = bacc.Bacc(trn_type="TRN2", target_bir_lowering=False, debug=False)
    dmae = lambda: getattr(nc, dma_engine)

    xT = nc.declare_dram_parameter("xT", [E, S], MD, isOutput=False)
    wq = nc.declare_dram_parameter("wq", [E, HD], MD, isOutput=False)
    wk = nc.declare_dram_parameter("wk", [E, HD], MD, isOutput=False)
    wv = nc.declare_dram_parameter("wv", [E, HD], MD, isOutput=False)
    wo = nc.declare_dram_parameter("wo", [HD, E], AD, isOutput=False)
    bq = nc.declare_dram_parameter("bq", [P, NPAIR], F32, isOutput=False)
    bk = nc.declare_dram_parameter("bk", [P, NPAIR], F32, isOutput=False)
    bvb = nc.declare_dram_parameter("bvb", [P, HD], F32, isOutput=False)
    out = nc.declare_dram_parameter("out", [S, E], AD, isOutput=True)
    rcp_dram = nc.dram_tensor("rcp_scratch", [HPC, S], F32)

    Exp = mybir.ActivationFunctionType.Exp
    Add = mybir.AluOpType.add

    xT_t = xT.rearrange("(kt p) s -> kt p s", p=P)

    with ExitStack() as ctx:
        tc = ctx.enter_context(tile.TileContext(nc))
        for _rep in range(reps):
            rctx = ctx.enter_context(ExitStack())
            const = rctx.enter_context(tc.tile_pool(name="const", bufs=1))
            proj = rctx.enter_context(tc.tile_pool(name="proj", bufs=1))
            attn_out = rctx.enter_context(tc.tile_pool(name="attn_out", bufs=1))

            bq_sb = const.tile([P, NPAIR], F32)
            bk_sb = const.tile([P, NPAIR], F32)
            bv_sb = const.tile([P, HD], F32)
            wo_sb = const.tile([DH, HPC, E], AD)

            # persistent activation tensors, per head pair
            qt_sb = [proj.tile([P, S], MD, name=f"qt{p}", tag=f"qt{p}")
                     for p in range(NPAIR)]
            kt_sb = [proj.tile([P, S], MD, name=f"kt{p}", tag=f"kt{p}")
                     for p in range(NPAIR)]
            vhat = [proj.tile([P, SK, 2 * VN], AD, name=f"vh{p}", tag=f"vh{p}")
                    for p in range(NPAIR)]
            outT = attn_out.tile([DH, HPC, S], AD)

            # ---- attention-phase pools (live through W1 + W2; opened
            # first so the projection pools can close before op_ps opens) ----
            actx = ExitStack()
            sc_ps = actx.enter_context(
                tc.tile_pool(name="sc_ps", bufs=2, space="PSUM"))
            pv_ps = actx.enter_context(
                tc.tile_pool(name="pv_ps", bufs=2, space="PSUM"))
            at_pool = actx.enter_context(tc.tile_pool(name="at", bufs=8))
            rcp_pool = actx.enter_context(tc.tile_pool(name="rcp", bufs=2))
            rb_pool = actx.enter_context(tc.tile_pool(name="rb", bufs=2))

            # ---- projection-phase pools (live through W1) ----
            pctx = ExitStack()
            xt_pool = pctx.enter_context(tc.tile_pool(name="xt", bufs=1))
            w_pool = pctx.enter_context(tc.tile_pool(name="wghts", bufs=1))
            prj_ps = pctx.enter_context(
                tc.tile_pool(name="prj_ps", bufs=2, space="PSUM"))

            # critical-path DMA order: wk first, xt tiles next (they gate
            # the K projection), then everything else
            wk_sb = w_pool.tile([P, EK, HD], MD)
            dmae().dma_start(
                out=wk_sb[:], in_=wk.rearrange("(kt p) n -> p kt n", p=P))
            wq_sb = w_pool.tile([P, EK, HD], MD)
            wv_sb = w_pool.tile([P, EK, HD], MD)

            def load_weights_rest():
                dmae().dma_start(out=bq_sb[:], in_=bq[:, :])
                dmae().dma_start(out=bk_sb[:], in_=bk[:, :])
                dmae().dma_start(out=bv_sb[:], in_=bvb[:, :])
                dmae().dma_start(
                    out=wq_sb[:], in_=wq.rearrange("(kt p) n -> p kt n", p=P))
                dmae().dma_start(
                    out=wv_sb[:], in_=wv.rearrange("(kt p) n -> p kt n", p=P))
                dmae().dma_start(
                    out=wo_sb[:], in_=wo.rearrange("(h p) e -> p h e", p=DH))

            xt = xt_pool.tile([P, EK, S], MD)

            def load_xt():
                # per-kt descriptors so the K projection can race the DMA
                for kt in range(EK):
                    dmae().dma_start(out=xt[:, kt, :], in_=xT_t[kt])

            def emit_qk_proj(p, w_sb, b_sb, dst, mbs):
                # psum[ch=128, MMW] = W_pair^T x xT block; bias on copy-out
                csl = slice(p * PW, (p + 1) * PW)
                for mb in mbs:
                    ssl = slice(mb * MMW, (mb + 1) * MMW)
                    ps = prj_ps.tile([P, MMW], F32, tag="prj")
                    for kt in range(EK):
                        nc.tensor.matmul(
                            ps[:],
                            lhsT=w_sb[:, kt, csl],
                            rhs=xt[:, kt, ssl],
                            start=(kt == 0),
                            stop=(kt == EK - 1),
                        )
                    nc.vector.tensor_scalar(
                        out=dst[:, ssl], in0=ps[:],
                        scalar1=b_sb[:, p:p + 1], scalar2=None, op0=Add,
                    )

            def emit_k_proj_ktouter(p, mb0s):
                # kt-outer K-projection: consumes xt tiles in DMA order
                csl = slice(p * PW, (p + 1) * PW)
                for mb0 in mb0s:
                    kps = [prj_ps.tile([P, MMW], F32, name=f"kp{mb0 + i}",
                                       tag="prj") for i in range(2)]
                    for kt in range(EK):
                        for i in range(2):
                            mb = mb0 + i
                            nc.tensor.matmul(
                                kps[i][:],
                                lhsT=wk_sb[:, kt, csl],
                                rhs=xt[:, kt, mb * MMW:(mb + 1) * MMW],
                                start=(kt == 0),
                                stop=(kt == EK - 1),
                            )
                    for i in range(2):
                        mb = mb0 + i
                        nc.vector.tensor_scalar(
                            out=kt_sb[p][:, mb * MMW:(mb + 1) * MMW],
                            in0=kps[i][:],
                            scalar1=bk_sb[:, p:p + 1], scalar2=None, op0=Add,
                        )

            def emit_v_proj(p, sts):
                # psum[s_tile=128, 128] = xT_tile^T x Wv_pair -> vhat columns
                csl = slice(p * PW, (p + 1) * PW)
                for st in sts:
                    ps = prj_ps.tile([P, PW], F32, tag="prj")
                    for kt in range(EK):
                        nc.tensor.matmul(
                            ps[:],
                            lhsT=xt[:, kt, st * P:(st + 1) * P],
                            rhs=wv_sb[:, kt, csl],
                            start=(kt == 0),
                            stop=(kt == EK - 1),
                        )
                    vdst = vhat[p][:, st, :].rearrange(
                        "p (h c) -> p h c", c=VN)[:, :, 0:DH]
                    nc.vector.tensor_add(
                        out=vdst,
                        in0=ps[:].rearrange("p (h c) -> p h c", c=DH),
                        in1=bv_sb[:, csl].rearrange("p (h c) -> p h c", c=DH),
                    )

            def emit_attn_qb(p, qb, extra_kt=None):
                qsl = slice(qb * QB, (qb + 1) * QB)
                pv0 = pv_ps.tile([VN, QB], F32, tag="pv")
                pv1 = pv_ps.tile([VN, QB], F32, tag="pv")

                def emit_pv(kt, at):
                    for hl, pv, asl in (
                        (0, pv0, slice(0, QB)),
                        (1, pv1, slice(QB, 2 * QB)),
                    ):
                        nc.tensor.matmul(
                            pv[:, :],
                            lhsT=vhat[p][:, kt, hl * VN:(hl + 1) * VN],
                            rhs=at[:, asl],
                            start=(kt == 0),
                            stop=(kt == SK - 1),
                        )

                # PV lags one kt so the in-order PE stream never parks on the
                # freshly-issued exp: sc(kt+1) issues right after pv(kt-1),
                # keeping the ACT queue fed regardless of semaphore latency.
                pending = None
                for kt in range(SK):
                    if extra_kt is not None:
                        extra_kt(kt)
                    ksl = slice(kt * P, (kt + 1) * P)
                    sc = sc_ps.tile([P, 2 * QB], F32, tag="sc")
                    nc.tensor.matmul(
                        sc[:, 0:QB],
                        lhsT=kt_sb[p][0:DH, ksl],
                        rhs=qt_sb[p][0:DH, qsl],
                        start=True, stop=True,
                    )
                    nc.tensor.matmul(
                        sc[:, QB:2 * QB],
                        lhsT=kt_sb[p][DH:P, ksl],
                        rhs=qt_sb[p][DH:P, qsl],
                        start=True, stop=True,
                    )
                    at = at_pool.tile([P, 2 * QB], AD, tag="at")
                    nc.scalar.activation(out=at[:], in_=sc[:], func=Exp)
                    if pending is not None:
                        emit_pv(*pending)
                    pending = (kt, at)
                emit_pv(*pending)
                for hl, pv in ((0, pv0), (1, pv1)):
                    h = 2 * p + hl
                    rcp = rcp_pool.tile([VN, QB], F32, tag="rcp")
                    nc.vector.reciprocal(out=rcp[DH:VN, :], in_=pv[DH:VN, :])
                    nc.vector.tensor_copy(
                        out=outT[0:DH, h, qsl], in_=pv[0:DH, :])
                    dmae().dma_start(
                        out=rcp_dram[h:h + 1, qsl],
                        in_=rcp[DH:VN, :],
                    )
                    rb = rb_pool.tile([DH, QB], F32, tag="rb")
                    dmae().dma_start(
                        out=rb[:],
                        in_=rcp_dram[h:h + 1, qsl].to_broadcast((DH, QB)),
                    )
                    nc.vector.tensor_mul(
                        out=outT[:, h, qsl], in0=outT[:, h, qsl],
                        in1=rb[:],
                    )

            # ====== prologue: K(p0) kt-outer racing the xt DMA, Q(p0,qb0) ==
            load_xt()
            load_weights_rest()
            for p in range(NPAIR):
                ones_ap = vhat[p][:].rearrange(
                    "p st (h c) -> p st h c", c=VN)[:, :, :, DH:VN]
                nc.vector.memset(ones_ap, 1.0)
            # Q(qb0) between the K groups: scores(qb0, kt<8) need only K
            # mb0/mb1, so the first exp can fire while K-group-2 still runs
            emit_k_proj_ktouter(0, [0])
            emit_qk_proj(0, wq_sb, bq_sb, qt_sb[0], [0])
            emit_k_proj_ktouter(0, [2])

            def v1_first_half(kt):
                if kt < SK // 2:
                    emit_v_proj(1, [kt])

            def v1_second_half(kt):
                if kt >= SK // 2:
                    emit_v_proj(1, [kt])

            # ========== W1: attn(p0) + projections for p0-rest and p1 ======
            for qb in range(NQB):
                if qb == 0:
                    emit_attn_qb(0, qb, extra_kt=lambda kt: emit_v_proj(0, [kt]))
                    emit_qk_proj(0, wq_sb, bq_sb, qt_sb[0], range(1, NMB))
                elif qb == 1:
                    emit_attn_qb(0, qb)
                    emit_qk_proj(1, wk_sb, bk_sb, kt_sb[1], range(NMB))
                elif qb == 2:
                    emit_attn_qb(0, qb, extra_kt=v1_first_half)
                else:
                    emit_attn_qb(0, qb)
                    emit_qk_proj(1, wq_sb, bq_sb, qt_sb[1], range(NMB))

            # ========== W2: attn(p1) + out-proj (all heads, lagged) ========
            # V(p1) second half projects inside W2-qb0 (no out-proj there
            # yet); the projection pools close right after so op_ps can open.
            emit_attn_qb(1, 0, extra_kt=v1_second_half)
            pctx.close()

            octx = ExitStack()
            op_ps = octx.enter_context(
                tc.tile_pool(name="op_ps", bufs=2, space="PSUM"))
            ob_pool = octx.enter_context(tc.tile_pool(name="ob", bufs=2))

            def emit_op_qb(qb):
                # one ob staging tile + one store DMA per qb; contraction
                # over all 4 heads (K=64 each) accumulated in psum
                ob = ob_pool.tile([P, QB // P, E], AD, tag="ob")
                for m in range(QB // P):
                    msl = slice(qb * QB + m * P, qb * QB + (m + 1) * P)
                    for nb in range(NEB):
                        esl = slice(nb * EMW, (nb + 1) * EMW)
                        ps = op_ps.tile([P, EMW], F32, tag="op")
                        for h in range(HPC):
                            nc.tensor.matmul(
                                ps[:],
                                lhsT=outT[:, h, msl],
                                rhs=wo_sb[:, h, esl],
                                start=(h == 0),
                                stop=(h == HPC - 1),
                            )
                        nc.vector.tensor_copy(out=ob[:, m, esl], in_=ps[:])
                dst = out[qb * QB:(qb + 1) * QB, :].rearrange(
                    "(m p) e -> p m e", p=P)
                dmae().dma_start(out=dst, in_=ob[:])

            for qb in range(1, NQB):
                emit_attn_qb(1, qb)
                emit_op_qb(qb - 1)
            emit_op_qb(NQB - 1)

            octx.close()
            actx.close()
            rctx.close()

    nc.compile()
    return nc


def make_in_maps(x, Wq, bq, Wk, bk, Wv, bv, Wo, n_cores=N_CORES,
                 mm_np_dtype=np.float32, wo_np_dtype=None):
    """Host-side sharding: per-core input dict list."""
    x = np.asarray(x, dtype=np.float32)
    B = x.shape[0]
    groups = n_cores // B
    scale = 1.0 / np.sqrt(np.float32(DH))
    if wo_np_dtype is None:
        wo_np_dtype = mm_np_dtype
    in_maps = []
    for c in range(n_cores):
        b, g = divmod(c, groups)
        hs = slice(g * HD, (g + 1) * HD)
        bq_s = (np.asarray(bq)[hs] * scale).astype(np.float32)
        bk_s = np.asarray(bk)[hs].astype(np.float32)
        in_maps.append({
            "xT": np.ascontiguousarray(x[b].T).astype(mm_np_dtype),
            "wq": np.ascontiguousarray(
                np.asarray(Wq)[:, hs] * scale).astype(mm_np_dtype),
            "wk": np.ascontiguousarray(np.asarray(Wk)[:, hs]).astype(mm_np_dtype),
            "wv": np.ascontiguousarray(np.asarray(Wv)[:, hs]).astype(mm_np_dtype),
            "wo": np.ascontiguousarray(np.asarray(Wo)[hs, :]).astype(wo_np_dtype),
            "bq": np.ascontiguousarray(bq_s.reshape(-1, P).T),
            "bk": np.ascontiguousarray(bk_s.reshape(-1, P).T),
            "bvb": np.ascontiguousarray(
                np.broadcast_to(np.asarray(bv)[hs], (P, HD))
            ),
        })
    return in_maps


_NC_CACHE = {}


def _get_nc():
    if "nc" not in _NC_CACHE:
        _NC_CACHE["nc"] = build_nc(mm_dtype=mybir.dt.bfloat16,
                                   att_dtype=mybir.dt.bfloat16)
    return _NC_CACHE["nc"]


def kernel(x, Wq, bq, Wk, bk, Wv, bv, Wo, bo, _trace=False, _trace_kwargs=None):
    import ml_dtypes
    from concourse.bass_utils import run_bass_kernel_spmd

    x = np.asarray(x, dtype=np.float32)
    B, S, E = x.shape
    nc = _get_nc()
    in_maps = make_in_maps(x, Wq, bq, Wk, bk, Wv, bv, Wo,
                           mm_np_dtype=ml_dtypes.bfloat16,
                           wo_np_dtype=ml_dtypes.bfloat16)
    res = run_bass_kernel_spmd(
        nc, in_maps, list(range(N_CORES)),
        trace=_trace, **(_trace_kwargs or {}),
    )
    groups = N_CORES // B
    full = np.zeros((B, S, E), dtype=np.float64)
    for c in range(N_CORES):
        full[c // groups] += np.asarray(res.results[c]["out"], dtype=np.float64)
    full += np.asarray(bo, dtype=np.float64)
    out = full.astype(np.float32)
    if _trace:
        return out, res
    return out
